# revision 18
# baseline (speedup 1.0000x reference)
"""Multi-head attention (B=2, S=2048, H=1024, 16 heads) on 8 TRN2 NeuronCores.

Sharding: tensor-parallel over heads. Each core owns 2 heads (128 of the 1024
q/k/v projection dims) and computes, for all 4096 tokens:
  QT = (Wq_local @ x^T) / sqrt(d)   (head-dim on partitions, tokens free)
  KT = Wk_local @ x^T, stored as TWO zero-padded per-head copies so the
       scores matmuls contract over a full K=128 (K=64 matmuls stay clock-
       gated at 1.2 GHz on this silicon; the zero rows annihilate the other
       head's Q rows)
  V  = x @ Wv_local^T (tokens on partitions) + a per-head ones column
  per (batch, 1024-wide q-group, head), software-pipelined by one k-tile:
       scores^T psum tile -> exp on ScalarE (direct from PSUM, bf16 out)
       ctx^T accum [65, 1024] += V_aug^T @ exp^T  (row 64 = softmax denoms)
Unnormalized ctx + denominators (bf16) are exchanged with one AllToAll
(head-split -> token-split). Each core then holds all 16 heads' ctx rows for
its 512 tokens: one DVE reciprocal + a K=16 indicator matmul per 128-row
block broadcasts 1/denom, and the full output projection (bias folded in as
a K=1 matmul) produces complete fp32 rows. Host concatenates the 8 shards.

All matmuls run in bf16 (input staging casts on host); projections fold the
biases in via an appended ones row on x^T. End-to-end rel err ~4e-3.
"""

import os

import numpy as np

HIDDEN = 1024
HEADS = 16
D = 64  # head dim
B = 2
S = 2048
T = B * S  # 4096 tokens
NCORES = 8
HPC = HEADS // NCORES  # 2 heads per core
LOC = HPC * D  # 128 local projection dims per core
KP = 1152  # contraction dim padded: 1024 hidden + 1 bias row + 127 zeros
KSUB = KP // 128  # 9
TCHUNK = 512  # a2a token chunk (= tokens per core for out_proj)
QG = 1024  # q-group width in phase 2 (2 psum banks)

_cache = {}

TRACE = os.environ.get("KERNEL_TRACE", "0") == "1"
LAST = {}


def _build_program():
    import concourse.mybir as mybir
    import concourse.tile as tile
    from concourse import bacc
    from concourse.bass import ds, ts
    from contextlib import ExitStack

    dt = mybir.dt
    f32 = dt.float32
    f32r = dt.float32r
    bf16 = dt.bfloat16
    AF = mybir.ActivationFunctionType

    nc = bacc.Bacc(
        "TRN2",
        target_bir_lowering=False,
        debug=False,
        enable_asserts=False,
        num_devices=NCORES,
    )

    xT = nc.dram_tensor("xT", [KP, T], bf16, kind="ExternalInput").ap()
    wqT = nc.dram_tensor("wqT", [128, KSUB * LOC], bf16, kind="ExternalInput").ap()
    wkT = nc.dram_tensor("wkT", [128, KSUB * LOC], bf16, kind="ExternalInput").ap()
    wvT = nc.dram_tensor("wvT", [128, KSUB * LOC], bf16, kind="ExternalInput").ap()
    woT = nc.dram_tensor("woT", [HIDDEN, HIDDEN], bf16, kind="ExternalInput").ap()
    bo = nc.dram_tensor("bo", [1, HIDDEN], bf16, kind="ExternalInput").ap()
    ind = nc.dram_tensor("ind", [HEADS, NCORES * 128], bf16, kind="ExternalInput").ap()
    out = nc.dram_tensor("out", [TCHUNK, HIDDEN], f32, kind="ExternalOutput").ap()

    xT_r = xT.rearrange("(kt p) t -> p kt t", p=128)
    wqT_r = wqT.rearrange("p (kt m) -> p kt m", kt=KSUB)
    wkT_r = wkT.rearrange("p (kt m) -> p kt m", kt=KSUB)
    wvT_r = wvT.rearrange("p (kt m) -> p kt m", kt=KSUB)
    woT_r = woT.rearrange("(kt p) o -> p kt o", p=128)

    NT = T // TCHUNK  # 8 projection token tiles
    NKT = S // 128  # 16 key tiles per batch
    CROWS = LOC + HPC  # 130 rows per a2a chunk

    with tile.TileContext(nc) as tc, ExitStack() as stack:
        persist = stack.enter_context(tc.tile_pool(name="persist", bufs=1))
        QT_sb = persist.tile([128, T], bf16, name="QT_sb")
        KTp0 = persist.tile([128, T], bf16, name="KTp0")
        KTp1 = persist.tile([128, T], bf16, name="KTp1")
        Vaug = persist.tile([128, T // 128, 2 * (D + 1)], bf16, name="Vaug")
        woT_sb = persist.tile([128, 8, HIDDEN], bf16, name="woT_sb")
        bo_sb = persist.tile([1, HIDDEN], bf16, name="bo_sb")
        ones1 = persist.tile([1, 128], bf16, name="ones1")
        ind_sb = persist.tile([HEADS, NCORES * 128], bf16, name="ind_sb")

        nc.gpsimd.dma_start(woT_sb[:], woT_r)
        nc.gpsimd.dma_start(bo_sb[:], bo)
        nc.gpsimd.dma_start(ind_sb[:], ind)
        nc.any.memset(ones1[:], 1.0)
        nc.any.memset(KTp0[D:128, :], 0.0)
        nc.any.memset(KTp1[0:D, :], 0.0)
        nc.any.memset(Vaug[:, :, D : D + 1], 1.0)
        nc.any.memset(Vaug[:, :, 2 * D + 1 : 2 * D + 2], 1.0)

        dram = stack.enter_context(tc.tile_pool(name="dram", bufs=1, space="DRAM"))
        a2a_in = dram.tile([NCORES, CROWS, TCHUNK], bf16, name="a2a_in")
        a2a_out = dram.tile([NCORES, CROWS, TCHUNK], bf16, name="a2a_out")

        # ---- phase 1: q/k/v projections --------------------------------------
        with (
            tc.tile_pool(name="wpool", bufs=1) as wpool,
            tc.tile_pool(name="xpool", bufs=2) as xpool,
            tc.tile_pool(name="p1psum", bufs=3, space="PSUM") as p1psum,
            tc.tile_pool(name="p1vpsum", bufs=3, space="PSUM") as p1vpsum,
        ):
            wq_sb = wpool.tile([128, KSUB, LOC], bf16, name="wq_sb")
            wk_sb = wpool.tile([128, KSUB, LOC], bf16, name="wk_sb")
            wv_sb = wpool.tile([128, KSUB, LOC], bf16, name="wv_sb")
            nc.sync.dma_start(wq_sb[:], wqT_r)
            nc.sync.dma_start(wk_sb[:], wkT_r)
            nc.sync.dma_start(wv_sb[:], wvT_r)

            for tci in range(4):
                xt = xpool.tile([128, KSUB, 1024], bf16, name="xt", tag="xt")
                for ktg in range(3):
                    nc.sync.dma_start(
                        xt[:, 3 * ktg : 3 * ktg + 3, :],
                        xT_r[:, 3 * ktg : 3 * ktg + 3, ts(tci, 1024)],
                    )

                for w_sb, nm in ((wq_sb, "q"), (wk_sb, "k")):
                    for nh2 in range(2):
                        ps = p1psum.tile([128, 512], f32, name=f"ps_{nm}", tag="qk")
                        for kt in range(KSUB):
                            nc.tensor.matmul(
                                ps[:],
                                lhsT=w_sb[:, kt, :],
                                rhs=xt[:, kt, ds(nh2 * 512, 512)],
                                start=(kt == 0),
                                stop=(kt == KSUB - 1),
                            )
                        tsl = ds(tci * 1024 + nh2 * 512, 512)
                        if nm == "q":
                            nc.vector.tensor_copy(QT_sb[:, tsl], ps[:])
                        else:
                            nc.vector.tensor_copy(KTp0[0:D, tsl], ps[0:D, :])
                            nc.vector.tensor_copy(KTp1[D:128, tsl], ps[D:128, :])

                for sub in range(8):
                    psv = p1vpsum.tile([128, LOC], f32, name="psv", tag="v")
                    for kt in range(KSUB):
                        nc.tensor.matmul(
                            psv[:],
                            lhsT=xt[:, kt, ts(sub, 128)],
                            rhs=wv_sb[:, kt, :],
                            start=(kt == 0),
                            stop=(kt == KSUB - 1),
                        )
                    tchunk4 = tci * 8 + sub
                    nc.vector.tensor_copy(Vaug[:, tchunk4, 0:D], psv[:, 0:D])
                    nc.vector.tensor_copy(
                        Vaug[:, tchunk4, D + 1 : 2 * D + 1], psv[:, D : 2 * D]
                    )

        # ---- phase 2: attention per (batch, q-group, head) -------------------
        with (
            tc.tile_pool(name="spsum", bufs=2, space="PSUM") as spsum,
            tc.tile_pool(name="cpsum", bufs=2, space="PSUM") as cpsum,
            tc.tile_pool(name="epool", bufs=4) as epool,
            tc.tile_pool(name="cupool", bufs=2) as cupool,
        ):
            for b in range(B):
                for qg in range(S // QG):
                    q0 = b * S + qg * QG
                    ps_ctx = [
                        cpsum.tile([D + 1, QG], f32, name=f"ps_ctx{h}", tag="ctx")
                        for h in range(HPC)
                    ]
                    ets = {}

                    def emit_scores(kt):
                        k0 = b * S + kt * 128
                        for h in range(HPC):
                            KTp = KTp0 if h == 0 else KTp1
                            ps_s = spsum.tile([128, QG], f32, name="ps_s", tag="s")
                            for half in range(QG // 512):
                                nc.tensor.matmul(
                                    ps_s[:, ts(half, 512)],
                                    lhsT=KTp[:, ds(k0, 128)],
                                    rhs=QT_sb[:, ds(q0 + half * 512, 512)],
                                    start=True,
                                    stop=True,
                                )
                            et = epool.tile([128, QG], bf16, name="et", tag="e")
                            nc.scalar.activation(et[:], ps_s[:], AF.Exp)
                            ets[(kt, h)] = et

                    def emit_ctx(kt):
                        for h in range(HPC):
                            va = h * (D + 1)
                            et = ets.pop((kt, h))
                            for half in range(QG // 512):
                                nc.tensor.matmul(
                                    ps_ctx[h][:, ts(half, 512)],
                                    lhsT=Vaug[:, b * NKT + kt, va : va + D + 1],
                                    rhs=et[:, ts(half, 512)],
                                    start=(kt == 0),
                                    stop=(kt == NKT - 1),
                                    skip_group_check=True,
                                )

                    for kt in range(NKT):
                        emit_scores(kt)
                        if kt > 0:
                            emit_ctx(kt - 1)
                    emit_ctx(NKT - 1)
                    for h in range(HPC):
                        ctxu = cupool.tile([D + 1, QG], bf16, name="ctxu", tag="cu")
                        nc.vector.tensor_copy(ctxu[:], ps_ctx[h][:])
                        for half in range(QG // 512):
                            j = (q0 + half * 512) // TCHUNK
                            nc.sync.dma_start(
                                a2a_in[j, h * (D + 1) : (h + 1) * (D + 1), :],
                                ctxu[:, ts(half, 512)],
                            )

        # ---- phase 3: all-to-all re-shard (head-split -> token-split) --------
        nc.gpsimd.collective_compute(
            "AllToAll",
            mybir.AluOpType.bypass,
            replica_groups=[list(range(NCORES))],
            ins=[a2a_in.opt()],
            outs=[a2a_out.opt()],
        )

        # ---- phase 4: normalize + output projection on my 512 tokens --------
        with (
            tc.tile_pool(name="ctpool", bufs=1) as ctpool,
            tc.tile_pool(name="bcpsum", bufs=2, space="PSUM") as bcpsum,
            tc.tile_pool(name="opsum", bufs=2, space="PSUM") as opsum,
            tc.tile_pool(name="obuf", bufs=2) as obuf,
        ):
            ct_sb = ctpool.tile([128, NCORES, TCHUNK], bf16, name="ct_sb")
            sums16 = ctpool.tile([HEADS, TCHUNK], bf16, name="sums16")
            recip16 = ctpool.tile([HEADS, TCHUNK], bf16, name="recip16")
            for i in range(NCORES):
                blk = a2a_out[i].rearrange("(h e) t -> h e t", e=D + 1)
                nc.sync.dma_start(
                    sums16[HPC * i : HPC * (i + 1), :], blk[:, D : D + 1, :]
                )
            for i in range(NCORES):
                blk = a2a_out[i].rearrange("(h e) t -> h e t", e=D + 1)
                nc.sync.dma_start(ct_sb[:, i, :], blk[:, 0:D, :])
            with nc.allow_low_precision(reason="softmax denom reciprocal in bf16"):
                nc.vector.reciprocal(recip16[:], sums16[:])
            ps_w = bcpsum.tile([128, TCHUNK], f32, name="ps_w", tag="warm")
            for _ in range(18):
                nc.tensor.matmul(
                    ps_w[:],
                    lhsT=ct_sb[:, 0, 0:128],
                    rhs=ct_sb[:, 0, :],
                    start=True,
                    stop=True,
                )
            for i in range(NCORES):
                ps_bc = bcpsum.tile([128, TCHUNK], f32, name="ps_bc", tag="bc")
                nc.tensor.matmul(
                    ps_bc[:],
                    lhsT=ind_sb[:, ts(i, 128)],
                    rhs=recip16[:],
                    start=True,
                    stop=True,
                )
                nc.vector.tensor_tensor(
                    ct_sb[:, i, :], ct_sb[:, i, :], ps_bc[:], mybir.AluOpType.mult
                )
            for tc4 in range(TCHUNK // 128):
                for nh in range(2):
                    pso = opsum.tile([128, 512], f32, name="pso", tag="o")
                    for kt in range(8):
                        nc.tensor.matmul(
                            pso[:],
                            lhsT=ct_sb[:, kt, ts(tc4, 128)],
                            rhs=woT_sb[:, kt, ds(nh * 512, 512)],
                            start=(kt == 0),
                            stop=False,
                        )
                    nc.tensor.matmul(
                        pso[:],
                        lhsT=ones1[0:1, 0:128],
                        rhs=bo_sb[0:1, ds(nh * 512, 512)],
                        start=False,
                        stop=True,
                    )
                    ob = obuf.tile([128, 512], f32, name="ob", tag="ob")
                    nc.vector.tensor_copy(ob[:], pso[:])
                    nc.sync.dma_start(out[ts(tc4, 128), ds(nh * 512, 512)], ob[:])

    nc.compile()
    return nc


def kernel(x, wq, bq, wk, bk, wv, bv, wo, bo):
    import ml_dtypes
    from concourse import bass_utils

    if "nc" not in _cache:
        _cache["nc"] = _build_program()
    nc = _cache["nc"]

    x = np.asarray(x, np.float32).reshape(T, HIDDEN)
    wq = np.asarray(wq, np.float32)
    bq = np.asarray(bq, np.float32)
    wk = np.asarray(wk, np.float32)
    bk = np.asarray(bk, np.float32)
    wv = np.asarray(wv, np.float32)
    bv = np.asarray(bv, np.float32)
    wo = np.asarray(wo, np.float32)
    bo = np.asarray(bo, np.float32)

    xT_aug = np.zeros((KP, T), np.float32)
    xT_aug[:HIDDEN] = x.T
    xT_aug[HIDDEN] = 1.0
    xT_aug = xT_aug.astype(ml_dtypes.bfloat16)

    scale = 1.0 / np.sqrt(D)
    woT_full = np.ascontiguousarray(wo.T).astype(ml_dtypes.bfloat16)
    bo_row = np.ascontiguousarray(bo[None, :]).astype(ml_dtypes.bfloat16)

    # ind[r, i*128 + m] = 1 where head r owns row m of core i's ctx block
    ind_np = np.zeros((HEADS, NCORES * 128), np.float32)
    for i in range(NCORES):
        for m in range(128):
            ind_np[HPC * i + m // D, i * 128 + m] = 1.0
    ind_np = ind_np.astype(ml_dtypes.bfloat16)

    def wt_aug(w, b, s):
        m = np.zeros((KP, LOC), np.float32)
        m[:HIDDEN] = (w * s).T
        m[HIDDEN] = b * s
        m = m.reshape(KSUB, 128, LOC).transpose(1, 0, 2).reshape(128, KSUB * LOC)
        return np.ascontiguousarray(m).astype(ml_dtypes.bfloat16)

    in_maps = []
    for c in range(NCORES):
        rows = slice(LOC * c, LOC * (c + 1))
        in_maps.append(
            {
                "xT": xT_aug,
                "wqT": wt_aug(wq[rows], bq[rows], scale),
                "wkT": wt_aug(wk[rows], bk[rows], 1.0),
                "wvT": wt_aug(wv[rows], bv[rows], 1.0),
                "woT": woT_full,
                "bo": bo_row,
                "ind": ind_np,
            }
        )

    res = bass_utils.run_bass_kernel_spmd(
        nc, in_maps, core_ids=list(range(NCORES)), trace=TRACE
    )
    LAST["result"] = res
    LAST["exec_time_ns"] = res.exec_time_ns

    full = np.concatenate([res.results[c]["out"] for c in range(NCORES)], axis=0)
    return full.reshape(B, S, HIDDEN)


# revision 21
# speedup vs baseline: 1.0916x; 1.0916x over previous
"""Multi-head attention (B=2, S=2048, H=1024, 16 heads) on 8 TRN2 NeuronCores.

Sharding: tensor-parallel over heads. Each core owns 2 heads (128 of the 1024
q/k/v projection dims) and computes, for all 4096 tokens:
  QT = (Wq_local @ x^T) / sqrt(d)   (head-dim on partitions, tokens free)
  KT = Wk_local @ x^T, stored as TWO zero-padded per-head copies so the
       scores matmuls contract over a full K=128 (K=64 matmuls stay clock-
       gated at 1.2 GHz on this silicon; the zero rows annihilate the other
       head's Q rows)
  V  = x @ Wv_local^T (tokens on partitions) + a per-head ones column
  per (batch, 1024-wide q-group, head), software-pipelined by one k-tile:
       scores^T psum tile -> exp on ScalarE (direct from PSUM, bf16 out)
       ctx^T accum [65, 1024] += V_aug^T @ exp^T  (row 64 = softmax denoms)
Unnormalized ctx + denominators (bf16) are exchanged with one AllToAll
(head-split -> token-split). Each core then holds all 16 heads' ctx rows for
its 512 tokens: one DVE reciprocal + a K=16 indicator matmul per 128-row
block broadcasts 1/denom, and the full output projection (bias folded in as
a K=1 matmul) produces complete fp32 rows. Host concatenates the 8 shards.

All matmuls run in bf16 (input staging casts on host); projections fold the
biases in via an appended ones row on x^T. End-to-end rel err ~4e-3.
"""

import os

import numpy as np

HIDDEN = 1024
HEADS = 16
D = 64  # head dim
B = 2
S = 2048
T = B * S  # 4096 tokens
NCORES = 8
HPC = HEADS // NCORES  # 2 heads per core
LOC = HPC * D  # 128 local projection dims per core
KP = 1152  # contraction dim padded: 1024 hidden + 1 bias row + 127 zeros
KSUB = KP // 128  # 9
TCHUNK = 512  # a2a token chunk (= tokens per core for out_proj)
QG = 1024  # q-group width in phase 2 (2 psum banks)

_cache = {}

TRACE = os.environ.get("KERNEL_TRACE", "0") == "1"
LAST = {}


def _build_program():
    import concourse.mybir as mybir
    import concourse.tile as tile
    from concourse import bacc
    from concourse.bass import ds, ts
    from contextlib import ExitStack

    dt = mybir.dt
    f32 = dt.float32
    f32r = dt.float32r
    bf16 = dt.bfloat16
    AF = mybir.ActivationFunctionType

    nc = bacc.Bacc(
        "TRN2",
        target_bir_lowering=False,
        debug=False,
        enable_asserts=False,
        num_devices=NCORES,
    )

    xT = nc.dram_tensor("xT", [KP, T], bf16, kind="ExternalInput").ap()
    wqT = nc.dram_tensor("wqT", [128, KSUB * LOC], bf16, kind="ExternalInput").ap()
    wkT = nc.dram_tensor("wkT", [128, KSUB * LOC], bf16, kind="ExternalInput").ap()
    wvT = nc.dram_tensor("wvT", [128, KSUB * LOC], bf16, kind="ExternalInput").ap()
    woT = nc.dram_tensor("woT", [HIDDEN, HIDDEN], bf16, kind="ExternalInput").ap()
    bo = nc.dram_tensor("bo", [1, HIDDEN], bf16, kind="ExternalInput").ap()
    ind = nc.dram_tensor("ind", [HEADS, NCORES * 128], bf16, kind="ExternalInput").ap()
    out = nc.dram_tensor("out", [TCHUNK, HIDDEN], f32, kind="ExternalOutput").ap()

    xT_r = xT.rearrange("(kt p) t -> p kt t", p=128)
    wqT_r = wqT.rearrange("p (kt m) -> p kt m", kt=KSUB)
    wkT_r = wkT.rearrange("p (kt m) -> p kt m", kt=KSUB)
    wvT_r = wvT.rearrange("p (kt m) -> p kt m", kt=KSUB)
    woT_r = woT.rearrange("(kt p) o -> p kt o", p=128)

    NT = T // TCHUNK  # 8 projection token tiles
    NKT = S // 128  # 16 key tiles per batch
    CROWS = LOC + HPC  # 130 rows per a2a chunk

    with tile.TileContext(nc) as tc, ExitStack() as stack:
        persist = stack.enter_context(tc.tile_pool(name="persist", bufs=1))
        QT_sb = persist.tile([128, T], bf16, name="QT_sb")
        KTp0 = persist.tile([128, T], bf16, name="KTp0")
        KTp1 = persist.tile([128, T], bf16, name="KTp1")
        Vaug = persist.tile([128, T // 128, 2 * (D + 1)], bf16, name="Vaug")
        woT_sb = persist.tile([128, 8, HIDDEN], bf16, name="woT_sb")
        bo_sb = persist.tile([1, HIDDEN], bf16, name="bo_sb")
        ones1 = persist.tile([1, 128], bf16, name="ones1")
        ind_sb = persist.tile([HEADS, NCORES * 128], bf16, name="ind_sb")

        nc.gpsimd.dma_start(woT_sb[:], woT_r)
        nc.gpsimd.dma_start(bo_sb[:], bo)
        nc.gpsimd.dma_start(ind_sb[:], ind)
        nc.any.memset(ones1[:], 1.0)
        nc.any.memset(KTp0[D:128, :], 0.0)
        nc.any.memset(KTp1[0:D, :], 0.0)
        nc.any.memset(Vaug[:, :, D : D + 1], 1.0)
        nc.any.memset(Vaug[:, :, 2 * D + 1 : 2 * D + 2], 1.0)

        dram = stack.enter_context(tc.tile_pool(name="dram", bufs=1, space="DRAM"))
        a2a_in0 = dram.tile([NCORES, CROWS, 256], bf16, name="a2a_in0")
        a2a_out0 = dram.tile([NCORES, CROWS, 256], bf16, name="a2a_out0")
        a2a_in1 = dram.tile([NCORES, CROWS, 256], bf16, name="a2a_in1")
        a2a_out1 = dram.tile([NCORES, CROWS, 256], bf16, name="a2a_out1")
        ct_sb = persist.tile([128, NCORES, TCHUNK], bf16, name="ct_sb")
        sums16 = persist.tile([HEADS, TCHUNK], bf16, name="sums16")

        # ---- phase 1: q/k/v projections --------------------------------------
        with (
            tc.tile_pool(name="wpool", bufs=1) as wpool,
            tc.tile_pool(name="xpool", bufs=2) as xpool,
            tc.tile_pool(name="p1psum", bufs=3, space="PSUM") as p1psum,
            tc.tile_pool(name="p1vpsum", bufs=3, space="PSUM") as p1vpsum,
        ):
            wq_sb = wpool.tile([128, KSUB, LOC], bf16, name="wq_sb")
            wk_sb = wpool.tile([128, KSUB, LOC], bf16, name="wk_sb")
            wv_sb = wpool.tile([128, KSUB, LOC], bf16, name="wv_sb")
            nc.sync.dma_start(wq_sb[:], wqT_r)
            nc.sync.dma_start(wk_sb[:], wkT_r)
            nc.sync.dma_start(wv_sb[:], wvT_r)

            for tci in range(4):
                xt = xpool.tile([128, KSUB, 1024], bf16, name="xt", tag="xt")
                for ktg in range(3):
                    nc.sync.dma_start(
                        xt[:, 3 * ktg : 3 * ktg + 3, :],
                        xT_r[:, 3 * ktg : 3 * ktg + 3, ts(tci, 1024)],
                    )

                for w_sb, nm in ((wq_sb, "q"), (wk_sb, "k")):
                    for nh2 in range(2):
                        ps = p1psum.tile([128, 512], f32, name=f"ps_{nm}", tag="qk")
                        for kt in range(KSUB):
                            nc.tensor.matmul(
                                ps[:],
                                lhsT=w_sb[:, kt, :],
                                rhs=xt[:, kt, ds(nh2 * 512, 512)],
                                start=(kt == 0),
                                stop=(kt == KSUB - 1),
                            )
                        tsl = ds(tci * 1024 + nh2 * 512, 512)
                        if nm == "q":
                            nc.vector.tensor_copy(QT_sb[:, tsl], ps[:])
                        else:
                            nc.vector.tensor_copy(KTp0[0:D, tsl], ps[0:D, :])
                            nc.vector.tensor_copy(KTp1[D:128, tsl], ps[D:128, :])

                for sub in range(8):
                    psv = p1vpsum.tile([128, LOC], f32, name="psv", tag="v")
                    for kt in range(KSUB):
                        nc.tensor.matmul(
                            psv[:],
                            lhsT=xt[:, kt, ts(sub, 128)],
                            rhs=wv_sb[:, kt, :],
                            start=(kt == 0),
                            stop=(kt == KSUB - 1),
                        )
                    tchunk4 = tci * 8 + sub
                    nc.vector.tensor_copy(Vaug[:, tchunk4, 0:D], psv[:, 0:D])
                    nc.vector.tensor_copy(
                        Vaug[:, tchunk4, D + 1 : 2 * D + 1], psv[:, D : 2 * D]
                    )

        # ---- phase 2: attention per (batch, q-group, head) -------------------
        with (
            tc.tile_pool(name="spsum", bufs=2, space="PSUM") as spsum,
            tc.tile_pool(name="cpsum", bufs=2, space="PSUM") as cpsum,
            tc.tile_pool(name="epool", bufs=4) as epool,
            tc.tile_pool(name="cupool", bufs=2) as cupool,
        ):
            def emit_a2a(b):
                a2a_in_b, a2a_out_b = (
                    (a2a_in0, a2a_out0) if b == 0 else (a2a_in1, a2a_out1)
                )
                nc.gpsimd.collective_compute(
                    "AllToAll",
                    mybir.AluOpType.bypass,
                    replica_groups=[list(range(NCORES))],
                    ins=[a2a_in_b.opt()],
                    outs=[a2a_out_b.opt()],
                )
                for i in range(NCORES):
                    blk = a2a_out_b[i].rearrange("(h e) t -> h e t", e=D + 1)
                    nc.sync.dma_start(
                        sums16[HPC * i : HPC * (i + 1), ds(b * 256, 256)],
                        blk[:, D : D + 1, :],
                    )
                    nc.sync.dma_start(
                        ct_sb[:, i, ds(b * 256, 256)], blk[:, 0:D, :]
                    )

            for b in range(B):
                for qg in range(S // QG):
                    q0 = b * S + qg * QG
                    ps_ctx = [
                        cpsum.tile([D + 1, QG], f32, name=f"ps_ctx{h}", tag="ctx")
                        for h in range(HPC)
                    ]
                    ets = {}

                    def emit_scores(kt):
                        k0 = b * S + kt * 128
                        for h in range(HPC):
                            KTp = KTp0 if h == 0 else KTp1
                            ps_s = spsum.tile([128, QG], f32, name="ps_s", tag="s")
                            for half in range(QG // 512):
                                nc.tensor.matmul(
                                    ps_s[:, ts(half, 512)],
                                    lhsT=KTp[:, ds(k0, 128)],
                                    rhs=QT_sb[:, ds(q0 + half * 512, 512)],
                                    start=True,
                                    stop=True,
                                )
                            et = epool.tile([128, QG], bf16, name="et", tag="e")
                            nc.scalar.activation(et[:], ps_s[:], AF.Exp)
                            ets[(kt, h)] = et

                    def emit_ctx(kt):
                        for h in range(HPC):
                            va = h * (D + 1)
                            et = ets.pop((kt, h))
                            for half in range(QG // 512):
                                nc.tensor.matmul(
                                    ps_ctx[h][:, ts(half, 512)],
                                    lhsT=Vaug[:, b * NKT + kt, va : va + D + 1],
                                    rhs=et[:, ts(half, 512)],
                                    start=(kt == 0),
                                    stop=(kt == NKT - 1),
                                    skip_group_check=True,
                                )

                    for kt in range(NKT):
                        emit_scores(kt)
                        if kt > 0:
                            emit_ctx(kt - 1)
                    emit_ctx(NKT - 1)
                    a2a_in_b = a2a_in0 if b == 0 else a2a_in1
                    for h in range(HPC):
                        ctxu = cupool.tile([D + 1, QG], bf16, name="ctxu", tag="cu")
                        nc.vector.tensor_copy(ctxu[:], ps_ctx[h][:])
                        for qtr in range(QG // 256):
                            j = qg * (QG // 256) + qtr
                            nc.sync.dma_start(
                                a2a_in_b[j, h * (D + 1) : (h + 1) * (D + 1), :],
                                ctxu[:, ts(qtr, 256)],
                            )
                emit_a2a(b)

        # ---- phase 4: normalize + output projection on my 512 tokens --------
        with (
            tc.tile_pool(name="ctpool", bufs=1) as ctpool,
            tc.tile_pool(name="bcpsum", bufs=2, space="PSUM") as bcpsum,
            tc.tile_pool(name="opsum", bufs=2, space="PSUM") as opsum,
            tc.tile_pool(name="obuf", bufs=2) as obuf,
        ):
            recip16 = ctpool.tile([HEADS, TCHUNK], bf16, name="recip16")
            with nc.allow_low_precision(reason="softmax denom reciprocal in bf16"):
                nc.vector.reciprocal(recip16[:], sums16[:])
            ps_w = bcpsum.tile([128, 256], f32, name="ps_w", tag="warm")
            for _ in range(24):
                nc.tensor.matmul(
                    ps_w[:],
                    lhsT=ct_sb[:, 0, 0:128],
                    rhs=ct_sb[:, 0, 0:256],
                    start=True,
                    stop=True,
                )
            for i in range(NCORES):
                ps_bc = bcpsum.tile([128, TCHUNK], f32, name="ps_bc", tag="bc")
                nc.tensor.matmul(
                    ps_bc[:],
                    lhsT=ind_sb[:, ts(i, 128)],
                    rhs=recip16[:],
                    start=True,
                    stop=True,
                )
                nc.vector.tensor_tensor(
                    ct_sb[:, i, :], ct_sb[:, i, :], ps_bc[:], mybir.AluOpType.mult
                )
            for tc4 in range(TCHUNK // 128):
                for nh in range(2):
                    pso = opsum.tile([128, 512], f32, name="pso", tag="o")
                    for kt in range(8):
                        nc.tensor.matmul(
                            pso[:],
                            lhsT=ct_sb[:, kt, ts(tc4, 128)],
                            rhs=woT_sb[:, kt, ds(nh * 512, 512)],
                            start=(kt == 0),
                            stop=False,
                        )
                    nc.tensor.matmul(
                        pso[:],
                        lhsT=ones1[0:1, 0:128],
                        rhs=bo_sb[0:1, ds(nh * 512, 512)],
                        start=False,
                        stop=True,
                    )
                    ob = obuf.tile([128, 512], f32, name="ob", tag="ob")
                    nc.vector.tensor_copy(ob[:], pso[:])
                    nc.sync.dma_start(out[ts(tc4, 128), ds(nh * 512, 512)], ob[:])

    nc.compile()
    return nc


def kernel(x, wq, bq, wk, bk, wv, bv, wo, bo):
    import ml_dtypes
    from concourse import bass_utils

    if "nc" not in _cache:
        _cache["nc"] = _build_program()
    nc = _cache["nc"]

    x = np.asarray(x, np.float32).reshape(T, HIDDEN)
    wq = np.asarray(wq, np.float32)
    bq = np.asarray(bq, np.float32)
    wk = np.asarray(wk, np.float32)
    bk = np.asarray(bk, np.float32)
    wv = np.asarray(wv, np.float32)
    bv = np.asarray(bv, np.float32)
    wo = np.asarray(wo, np.float32)
    bo = np.asarray(bo, np.float32)

    xT_aug = np.zeros((KP, T), np.float32)
    xT_aug[:HIDDEN] = x.T
    xT_aug[HIDDEN] = 1.0
    xT_aug = xT_aug.astype(ml_dtypes.bfloat16)

    scale = 1.0 / np.sqrt(D)
    woT_full = np.ascontiguousarray(wo.T).astype(ml_dtypes.bfloat16)
    bo_row = np.ascontiguousarray(bo[None, :]).astype(ml_dtypes.bfloat16)

    # ind[r, i*128 + m] = 1 where head r owns row m of core i's ctx block
    ind_np = np.zeros((HEADS, NCORES * 128), np.float32)
    for i in range(NCORES):
        for m in range(128):
            ind_np[HPC * i + m // D, i * 128 + m] = 1.0
    ind_np = ind_np.astype(ml_dtypes.bfloat16)

    def wt_aug(w, b, s):
        m = np.zeros((KP, LOC), np.float32)
        m[:HIDDEN] = (w * s).T
        m[HIDDEN] = b * s
        m = m.reshape(KSUB, 128, LOC).transpose(1, 0, 2).reshape(128, KSUB * LOC)
        return np.ascontiguousarray(m).astype(ml_dtypes.bfloat16)

    in_maps = []
    for c in range(NCORES):
        rows = slice(LOC * c, LOC * (c + 1))
        in_maps.append(
            {
                "xT": xT_aug,
                "wqT": wt_aug(wq[rows], bq[rows], scale),
                "wkT": wt_aug(wk[rows], bk[rows], 1.0),
                "wvT": wt_aug(wv[rows], bv[rows], 1.0),
                "woT": woT_full,
                "bo": bo_row,
                "ind": ind_np,
            }
        )

    res = bass_utils.run_bass_kernel_spmd(
        nc, in_maps, core_ids=list(range(NCORES)), trace=TRACE
    )
    LAST["result"] = res
    LAST["exec_time_ns"] = res.exec_time_ns

    full = np.empty((T, HIDDEN), np.float32)
    for c in range(NCORES):
        o = res.results[c]["out"]
        full[256 * c : 256 * (c + 1)] = o[0:256]
        full[S + 256 * c : S + 256 * (c + 1)] = o[256:512]
    return full.reshape(B, S, HIDDEN)


# revision 23
# speedup vs baseline: 1.2660x; 1.1598x over previous
"""Multi-head attention (B=2, S=2048, H=1024, 16 heads) on 8 TRN2 NeuronCores.

Sharding: tensor-parallel over heads. Each core owns 2 heads (128 of the 1024
q/k/v projection dims) and computes, for all 4096 tokens:
  QT = (Wq_local @ x^T) / sqrt(d)   (head-dim on partitions, tokens free)
  KT = Wk_local @ x^T, stored as TWO zero-padded per-head copies so the
       scores matmuls contract over a full K=128 (K=64 matmuls stay clock-
       gated at 1.2 GHz on this silicon; the zero rows annihilate the other
       head's Q rows)
  V  = x @ Wv_local^T (tokens on partitions) + a per-head ones column
  per (batch, 1024-wide q-group, head), software-pipelined by one k-tile:
       scores^T psum tile -> exp on ScalarE (direct from PSUM, bf16 out)
       ctx^T accum [65, 1024] += V_aug^T @ exp^T  (row 64 = softmax denoms)
Unnormalized ctx + denominators (bf16) are exchanged with one AllToAll
(head-split -> token-split). Each core then holds all 16 heads' ctx rows for
its 512 tokens: one DVE reciprocal + a K=16 indicator matmul per 128-row
block broadcasts 1/denom, and the full output projection (bias folded in as
a K=1 matmul) produces complete fp32 rows. Host concatenates the 8 shards.

All matmuls run in bf16 (input staging casts on host); projections fold the
biases in via an appended ones row on x^T. End-to-end rel err ~4e-3.
"""

import os

import numpy as np

HIDDEN = 1024
HEADS = 16
D = 64  # head dim
B = 2
S = 2048
T = B * S  # 4096 tokens
NCORES = 8
HPC = HEADS // NCORES  # 2 heads per core
LOC = HPC * D  # 128 local projection dims per core
KP = 1152  # contraction dim padded: 1024 hidden + 1 bias row + 127 zeros
KSUB = KP // 128  # 9
TCHUNK = 512  # a2a token chunk (= tokens per core for out_proj)
QG = 1024  # q-group width in phase 2 (2 psum banks)

_cache = {}

TRACE = os.environ.get("KERNEL_TRACE", "0") == "1"
LAST = {}


def _build_program():
    import concourse.mybir as mybir
    import concourse.tile as tile
    from concourse import bacc
    from concourse.bass import ds, ts
    from contextlib import ExitStack

    dt = mybir.dt
    f32 = dt.float32
    f32r = dt.float32r
    bf16 = dt.bfloat16
    AF = mybir.ActivationFunctionType

    nc = bacc.Bacc(
        "TRN2",
        target_bir_lowering=False,
        debug=False,
        enable_asserts=False,
        num_devices=NCORES,
    )

    xT = nc.dram_tensor("xT", [KP, T], bf16, kind="ExternalInput").ap()
    wqT = nc.dram_tensor("wqT", [128, KSUB * LOC], bf16, kind="ExternalInput").ap()
    wkT = nc.dram_tensor("wkT", [128, KSUB * LOC], bf16, kind="ExternalInput").ap()
    wvT = nc.dram_tensor("wvT", [128, KSUB * LOC], bf16, kind="ExternalInput").ap()
    woT = nc.dram_tensor("woT", [HIDDEN, HIDDEN], bf16, kind="ExternalInput").ap()
    bo = nc.dram_tensor("bo", [1, HIDDEN], bf16, kind="ExternalInput").ap()
    ind = nc.dram_tensor("ind", [HEADS, NCORES * 128], bf16, kind="ExternalInput").ap()
    out = nc.dram_tensor("out", [TCHUNK, HIDDEN], f32, kind="ExternalOutput").ap()

    xT_r = xT.rearrange("(kt p) t -> p kt t", p=128)
    wqT_r = wqT.rearrange("p (kt m) -> p kt m", kt=KSUB)
    wkT_r = wkT.rearrange("p (kt m) -> p kt m", kt=KSUB)
    wvT_r = wvT.rearrange("p (kt m) -> p kt m", kt=KSUB)
    woT_r = woT.rearrange("(kt p) o -> p kt o", p=128)

    NT = T // TCHUNK  # 8 projection token tiles
    NKT = S // 128  # 16 key tiles per batch
    CROWS = LOC + HPC  # 130 rows per a2a chunk

    with tile.TileContext(nc) as tc, ExitStack() as stack:
        persist = stack.enter_context(tc.tile_pool(name="persist", bufs=1))
        QT_sb = persist.tile([128, T], bf16, name="QT_sb")
        KTp0 = persist.tile([128, T], bf16, name="KTp0")
        KTp1 = persist.tile([128, T], bf16, name="KTp1")
        Vaug = persist.tile([128, T // 128, 2 * (D + 1)], bf16, name="Vaug")
        woT_sb = persist.tile([128, 8, HIDDEN], bf16, name="woT_sb")
        bo_sb = persist.tile([1, HIDDEN], bf16, name="bo_sb")
        ones1 = persist.tile([1, 128], bf16, name="ones1")
        ind_sb = persist.tile([HEADS, NCORES * 128], bf16, name="ind_sb")

        nc.gpsimd.dma_start(woT_sb[:], woT_r)
        nc.gpsimd.dma_start(bo_sb[:], bo)
        nc.gpsimd.dma_start(ind_sb[:], ind)
        nc.any.memset(ones1[:], 1.0)
        nc.any.memset(KTp0[D:128, :], 0.0)
        nc.any.memset(KTp1[0:D, :], 0.0)
        nc.any.memset(Vaug[:, :, D : D + 1], 1.0)
        nc.any.memset(Vaug[:, :, 2 * D + 1 : 2 * D + 2], 1.0)

        dram = stack.enter_context(tc.tile_pool(name="dram", bufs=1, space="DRAM"))
        a2a_in0 = dram.tile([NCORES, CROWS, 256], bf16, name="a2a_in0")
        a2a_out0 = dram.tile([NCORES, CROWS, 256], bf16, name="a2a_out0")
        a2a_in1 = dram.tile([NCORES, CROWS, 256], bf16, name="a2a_in1")
        a2a_out1 = dram.tile([NCORES, CROWS, 256], bf16, name="a2a_out1")
        ct_sb = persist.tile([128, NCORES, TCHUNK], bf16, name="ct_sb")
        sums16 = persist.tile([HEADS, TCHUNK], bf16, name="sums16")

        # ---- phase 1: q/k/v projections --------------------------------------
        with (
            tc.tile_pool(name="wpool", bufs=1) as wpool,
            tc.tile_pool(name="xpool", bufs=2) as xpool,
            tc.tile_pool(name="p1psum", bufs=3, space="PSUM") as p1psum,
            tc.tile_pool(name="p1vpsum", bufs=3, space="PSUM") as p1vpsum,
        ):
            wq_sb = wpool.tile([128, KSUB, LOC], bf16, name="wq_sb")
            wk_sb = wpool.tile([128, KSUB, LOC], bf16, name="wk_sb")
            wv_sb = wpool.tile([128, KSUB, LOC], bf16, name="wv_sb")
            nc.sync.dma_start(wq_sb[:], wqT_r)
            nc.sync.dma_start(wk_sb[:], wkT_r)
            nc.sync.dma_start(wv_sb[:], wvT_r)

            for tci in range(4):
                xt = xpool.tile([128, KSUB, 1024], bf16, name="xt", tag="xt")
                for ktg in range(3):
                    nc.sync.dma_start(
                        xt[:, 3 * ktg : 3 * ktg + 3, :],
                        xT_r[:, 3 * ktg : 3 * ktg + 3, ts(tci, 1024)],
                    )

                for w_sb, nm in ((wq_sb, "q"), (wk_sb, "k")):
                    for nh2 in range(2):
                        ps = p1psum.tile([128, 512], f32, name=f"ps_{nm}", tag="qk")
                        for kt in range(KSUB):
                            nc.tensor.matmul(
                                ps[:],
                                lhsT=w_sb[:, kt, :],
                                rhs=xt[:, kt, ds(nh2 * 512, 512)],
                                start=(kt == 0),
                                stop=(kt == KSUB - 1),
                            )
                        tsl = ds(tci * 1024 + nh2 * 512, 512)
                        if nm == "q":
                            nc.vector.tensor_copy(QT_sb[:, tsl], ps[:])
                        else:
                            nc.vector.tensor_copy(KTp0[0:D, tsl], ps[0:D, :])
                            nc.vector.tensor_copy(KTp1[D:128, tsl], ps[D:128, :])

                for sub in range(8):
                    psv = p1vpsum.tile([128, LOC], f32, name="psv", tag="v")
                    for kt in range(KSUB):
                        nc.tensor.matmul(
                            psv[:],
                            lhsT=xt[:, kt, ts(sub, 128)],
                            rhs=wv_sb[:, kt, :],
                            start=(kt == 0),
                            stop=(kt == KSUB - 1),
                        )
                    tchunk4 = tci * 8 + sub
                    nc.vector.tensor_copy(Vaug[:, tchunk4, 0:D], psv[:, 0:D])
                    nc.vector.tensor_copy(
                        Vaug[:, tchunk4, D + 1 : 2 * D + 1], psv[:, D : 2 * D]
                    )

        # ---- phase 2: attention per (batch, q-group, head) -------------------
        with (
            tc.tile_pool(name="spsum", bufs=2, space="PSUM") as spsum,
            tc.tile_pool(name="cpsum", bufs=2, space="PSUM") as cpsum,
            tc.tile_pool(name="epool", bufs=4) as epool,
            tc.tile_pool(name="cupool", bufs=2) as cupool,
        ):
            def emit_a2a(b):
                a2a_in_b, a2a_out_b = (
                    (a2a_in0, a2a_out0) if b == 0 else (a2a_in1, a2a_out1)
                )
                nc.gpsimd.collective_compute(
                    "AllToAll",
                    mybir.AluOpType.bypass,
                    replica_groups=[list(range(NCORES))],
                    ins=[a2a_in_b.opt()],
                    outs=[a2a_out_b.opt()],
                )
                sums_src = a2a_out_b.rearrange("i (h e) t -> (i h) e t", e=D + 1)
                nc.sync.dma_start(
                    sums16[:, ds(b * 256, 256)], sums_src[:, D, :]
                )
                for i in range(NCORES):
                    blk = a2a_out_b[i].rearrange("(h e) t -> h e t", e=D + 1)
                    nc.sync.dma_start(
                        ct_sb[:, i, ds(b * 256, 256)], blk[:, 0:D, :]
                    )

            for b in range(B):
                for qg in range(S // QG):
                    q0 = b * S + qg * QG
                    ps_ctx = [
                        cpsum.tile([D + 1, QG], f32, name=f"ps_ctx{h}", tag="ctx")
                        for h in range(HPC)
                    ]
                    ets = {}

                    def emit_scores(kt):
                        k0 = b * S + kt * 128
                        for h in range(HPC):
                            KTp = KTp0 if h == 0 else KTp1
                            ps_s = spsum.tile([128, QG], f32, name="ps_s", tag="s")
                            for half in range(QG // 512):
                                nc.tensor.matmul(
                                    ps_s[:, ts(half, 512)],
                                    lhsT=KTp[:, ds(k0, 128)],
                                    rhs=QT_sb[:, ds(q0 + half * 512, 512)],
                                    start=True,
                                    stop=True,
                                )
                            et = epool.tile([128, QG], bf16, name="et", tag="e")
                            nc.scalar.activation(et[:], ps_s[:], AF.Exp)
                            ets[(kt, h)] = et

                    def emit_ctx(kt):
                        for h in range(HPC):
                            va = h * (D + 1)
                            et = ets.pop((kt, h))
                            for half in range(QG // 512):
                                nc.tensor.matmul(
                                    ps_ctx[h][:, ts(half, 512)],
                                    lhsT=Vaug[:, b * NKT + kt, va : va + D + 1],
                                    rhs=et[:, ts(half, 512)],
                                    start=(kt == 0),
                                    stop=(kt == NKT - 1),
                                    skip_group_check=True,
                                )

                    for kt in range(NKT):
                        emit_scores(kt)
                        if kt > 0:
                            emit_ctx(kt - 1)
                    emit_ctx(NKT - 1)
                    a2a_in_b = a2a_in0 if b == 0 else a2a_in1
                    for h in range(HPC):
                        ctxu = cupool.tile([D + 1, QG], bf16, name="ctxu", tag="cu")
                        nc.vector.tensor_copy(ctxu[:], ps_ctx[h][:])
                        for qtr in range(QG // 256):
                            j = qg * (QG // 256) + qtr
                            nc.sync.dma_start(
                                a2a_in_b[j, h * (D + 1) : (h + 1) * (D + 1), :],
                                ctxu[:, ts(qtr, 256)],
                            )
                emit_a2a(b)

        # ---- phase 4: normalize + output projection on my 512 tokens --------
        with (
            tc.tile_pool(name="ctpool", bufs=1) as ctpool,
            tc.tile_pool(name="bcpsum", bufs=2, space="PSUM") as bcpsum,
            tc.tile_pool(name="opsum", bufs=2, space="PSUM") as opsum,
            tc.tile_pool(name="obuf", bufs=2) as obuf,
        ):
            recip16 = ctpool.tile([HEADS, TCHUNK], bf16, name="recip16")
            for b in range(B):
                bs = ds(b * 256, 256)
                with nc.allow_low_precision(reason="softmax denom recip in bf16"):
                    nc.vector.reciprocal(recip16[:, bs], sums16[:, bs])
                if b == 1:
                    ps_w = bcpsum.tile([128, 256], f32, name="ps_w", tag="warm")
                    for _ in range(12):
                        nc.tensor.matmul(
                            ps_w[:],
                            lhsT=ct_sb[:, 0, 256:384],
                            rhs=ct_sb[:, 0, 256:512],
                            start=True,
                            stop=True,
                        )
                for i in range(NCORES):
                    ps_bc = bcpsum.tile([128, 256], f32, name="ps_bc", tag="bc")
                    nc.tensor.matmul(
                        ps_bc[:],
                        lhsT=ind_sb[:, ts(i, 128)],
                        rhs=recip16[:, bs],
                        start=True,
                        stop=True,
                    )
                    nc.vector.tensor_tensor(
                        ct_sb[:, i, bs], ct_sb[:, i, bs], ps_bc[:],
                        mybir.AluOpType.mult,
                    )
                for tc4 in range(2 * b, 2 * b + 2):
                    for nh in range(2):
                        pso = opsum.tile([128, 512], f32, name="pso", tag="o")
                        for kt in range(8):
                            nc.tensor.matmul(
                                pso[:],
                                lhsT=ct_sb[:, kt, ts(tc4, 128)],
                                rhs=woT_sb[:, kt, ds(nh * 512, 512)],
                                start=(kt == 0),
                                stop=False,
                            )
                        nc.tensor.matmul(
                            pso[:],
                            lhsT=ones1[0:1, 0:128],
                            rhs=bo_sb[0:1, ds(nh * 512, 512)],
                            start=False,
                            stop=True,
                        )
                        ob = obuf.tile([128, 512], f32, name="ob", tag="ob")
                        nc.vector.tensor_copy(ob[:], pso[:])
                        nc.sync.dma_start(
                            out[ts(tc4, 128), ds(nh * 512, 512)], ob[:]
                        )

    nc.compile()
    return nc


def kernel(x, wq, bq, wk, bk, wv, bv, wo, bo):
    import ml_dtypes
    from concourse import bass_utils

    if "nc" not in _cache:
        _cache["nc"] = _build_program()
    nc = _cache["nc"]

    x = np.asarray(x, np.float32).reshape(T, HIDDEN)
    wq = np.asarray(wq, np.float32)
    bq = np.asarray(bq, np.float32)
    wk = np.asarray(wk, np.float32)
    bk = np.asarray(bk, np.float32)
    wv = np.asarray(wv, np.float32)
    bv = np.asarray(bv, np.float32)
    wo = np.asarray(wo, np.float32)
    bo = np.asarray(bo, np.float32)

    xT_aug = np.zeros((KP, T), np.float32)
    xT_aug[:HIDDEN] = x.T
    xT_aug[HIDDEN] = 1.0
    xT_aug = xT_aug.astype(ml_dtypes.bfloat16)

    scale = 1.0 / np.sqrt(D)
    woT_full = np.ascontiguousarray(wo.T).astype(ml_dtypes.bfloat16)
    bo_row = np.ascontiguousarray(bo[None, :]).astype(ml_dtypes.bfloat16)

    # ind[r, i*128 + m] = 1 where head r owns row m of core i's ctx block
    ind_np = np.zeros((HEADS, NCORES * 128), np.float32)
    for i in range(NCORES):
        for m in range(128):
            ind_np[HPC * i + m // D, i * 128 + m] = 1.0
    ind_np = ind_np.astype(ml_dtypes.bfloat16)

    def wt_aug(w, b, s):
        m = np.zeros((KP, LOC), np.float32)
        m[:HIDDEN] = (w * s).T
        m[HIDDEN] = b * s
        m = m.reshape(KSUB, 128, LOC).transpose(1, 0, 2).reshape(128, KSUB * LOC)
        return np.ascontiguousarray(m).astype(ml_dtypes.bfloat16)

    in_maps = []
    for c in range(NCORES):
        rows = slice(LOC * c, LOC * (c + 1))
        in_maps.append(
            {
                "xT": xT_aug,
                "wqT": wt_aug(wq[rows], bq[rows], scale),
                "wkT": wt_aug(wk[rows], bk[rows], 1.0),
                "wvT": wt_aug(wv[rows], bv[rows], 1.0),
                "woT": woT_full,
                "bo": bo_row,
                "ind": ind_np,
            }
        )

    res = bass_utils.run_bass_kernel_spmd(
        nc, in_maps, core_ids=list(range(NCORES)), trace=TRACE
    )
    LAST["result"] = res
    LAST["exec_time_ns"] = res.exec_time_ns

    full = np.empty((T, HIDDEN), np.float32)
    for c in range(NCORES):
        o = res.results[c]["out"]
        full[256 * c : 256 * (c + 1)] = o[0:256]
        full[S + 256 * c : S + 256 * (c + 1)] = o[256:512]
    return full.reshape(B, S, HIDDEN)
